# revision 1
# baseline (speedup 1.0000x reference)
"""AffinityBasedAveraging Trainium2 kernel.

Computes, for affinities [B,9,H,W] and embedding [B,C,H,W]:
    w = softmax(affinities, axis=1)  (then redundant L1-normalize)
    out[b,c,y,x] = sum_k w[b,k,y,x] * embedding[b,c,clip(y+oy_k),clip(x+ox_k)]

Sharding: 8 cores = 4 batches x 2 H-halves. Each core gets its batch's
affinity slab [9,256,512] and a replicate-padded embedding slab
[16,258,514] (1-row halo top/bottom, 1-col halo left/right pre-clamped on
the host), producing out slab [16,256,512].

Per-core schedule (partition dim = output rows; x in chunks of XC=128 —
measured ~2x faster per-element on DVE than 256-wide chunks):
  for each (y-tile of 128 rows) x (x-chunk of XC cols):
    A   <- dma aff tile [128; 9,XC]
    X    = exp(A)                       (ScalarE)
    S    = sum_k X                      (VectorE reduce, k innermost)
    R    = 1/S                          (VectorE)
    W    = X * R  (bcast over k)        (VectorE)
    E_oy <- dma emb rows shifted oy in {-1,0,1}  [128; 16,XC+2]
    taps 0..5: acc = sum W_k (bcast over c) * E_{oy(k)} (VectorE mult/add)
    taps 6..8: products into fold tiles (VectorE mult), folded into acc by
               the otherwise-idle DMA engines (gpsimd dma_start accum_op=add)
    out <- dma acc
"""

import numpy as np

import bass_rust
import concourse.bass as bass
import concourse.mybir as mybir
import concourse.tile as tile
from concourse.bass_utils import run_bass_kernel_spmd

F32 = mybir.dt.float32
AF = mybir.ActivationFunctionType
OP = mybir.AluOpType
AX = mybir.AxisListType

B, C, H, W = 4, 16, 512, 512
K = 9
OFFSETS = [(-1, -1), (-1, 0), (-1, 1), (0, -1), (0, 0), (0, 1), (1, -1), (1, 0), (1, 1)]
N_CORES = 8
HH = H // 2          # rows per core (256)
YT = 128             # y-tile rows (partition dim)
XC = 128             # x-chunk cols

_wsplit_ctr = [0]


def _split_multi_waits(nc):
    """This container's walrus rejects >1 semaphore wait per instruction
    ("Too many sync wait commands"). Split extra waits into same-engine
    NoOp prefixes."""
    n = 0
    for f in nc.m.functions:
        for bb in f.blocks:
            insts = bb.instructions
            if not any(
                i.sync_info is not None and len(i.sync_info.on_wait or []) > 1
                for i in insts
            ):
                continue
            new = []
            for inst in insts:
                si = inst.sync_info
                waits = list(si.on_wait) if si is not None and si.on_wait else []
                if len(waits) > 1:
                    for w in waits[:-1]:
                        _wsplit_ctr[0] += 1
                        nop = mybir.InstNoOp(name=f"I-wsplit-{_wsplit_ctr[0]}")
                        nop.engine = inst.engine
                        nop.sync_info = bass_rust.SyncInfo(on_wait=[w], on_update=[])
                        new.append(nop)
                        n += 1
                    inst.sync_info = bass_rust.SyncInfo(
                        on_wait=[waits[-1]], on_update=list(si.on_update or [])
                    )
                new.append(inst)
            insts[:] = new
    return n


def build_nc(split_waits=True, reps=1, dma_folds=2, xc=None, e_shift_mode="hbm3x", dma_ksum=False, xe=None, ebufs=2):
    xc = XC if xc is None else xc
    xe = xc if xe is None else xe  # E-load width (bigger descriptors)
    nc = bass.Bass("TRN2", target_bir_lowering=False, debug=False, num_devices=N_CORES)
    aff = nc.declare_dram_parameter("aff", [K, HH, W], F32, isOutput=False)
    emb = nc.declare_dram_parameter("emb", [C, HH + 2, W + 2], F32, isOutput=False)
    out = nc.declare_dram_parameter("out", [C, HH, W], F32, isOutput=True)

    with tile.TileContext(nc) as tc:
        with (
            tc.tile_pool(name="p_a", bufs=2) as p_a,
            tc.tile_pool(name="p_x", bufs=1) as p_x,
            tc.tile_pool(name="p_e", bufs=ebufs) as p_e,
            tc.tile_pool(name="p_acc", bufs=2) as p_acc,
            tc.tile_pool(name="p_tmp", bufs=2) as p_tmp,
            tc.tile_pool(name="p_s", bufs=1) as p_s,
        ):
            for _rep in range(reps):
              for ty in range(HH // YT):
                for xh in range(W // xc):
                    ys, xs = ty * YT, xh * xc

                    A = p_a.tile([YT, K, xc], F32, tag="A")
                    nc.sync.dma_start(
                        out=A[:],
                        in_=aff[:, ys : ys + YT, xs : xs + xc].rearrange(
                            "k y x -> y k x"
                        ),
                    )
                    X = p_x.tile([YT, K, xc], F32, tag="X")
                    nc.scalar.activation(X[:], A[:], AF.Exp)
                    S = p_s.tile([YT, xc], F32, tag="S")
                    if dma_ksum:
                        # k-sum via same-partition DMA accumulates (frees DVE)
                        nc.gpsimd.dma_start(out=S[:], in_=X[:, 0, :])
                        for kk in range(1, K):
                            nc.gpsimd.dma_start(
                                out=S[:], in_=X[:, kk, :], accum_op=OP.add
                            )
                    else:
                        nc.vector.tensor_reduce(
                            S[:], X[:].rearrange("p k x -> p x k"), AX.X, OP.add
                        )
                    R = p_s.tile([YT, xc], F32, tag="R")
                    nc.vector.reciprocal(R[:], S[:])
                    Wt = p_x.tile([YT, K, xc], F32, tag="W")
                    nc.vector.tensor_tensor(
                        Wt[:], X[:], R[:, None, :].to_broadcast((YT, K, xc)), OP.mult
                    )

                    E = {}
                    if e_shift_mode == "hbm3x":
                        if xh % (xe // xc) == 0:
                            e_tiles = {}
                            xes = (xs // xe) * xe
                            for oy in (-1, 0, 1):
                                t = p_e.tile([YT, C, xe + 2], F32, tag=f"E{oy}")
                                rs = ys + oy + 1
                                nc.sync.dma_start(
                                    out=t[:],
                                    in_=emb[
                                        :, rs : rs + YT, xes : xes + xe + 2
                                    ].rearrange("c y x -> y c x"),
                                )
                                e_tiles[oy] = t
                            build_nc._e_tiles = e_tiles
                        eoff = xs % xe
                        E = {
                            oy: build_nc._e_tiles[oy][:, :, eoff : eoff + xc + 2]
                            for oy in (-1, 0, 1)
                        }
                    else:
                        # load the 128-row window once from HBM; derive the
                        # +-1-row variants by partition-shifted SBUF->SBUF DMA
                        # copies plus a single-row HBM edge fill. Cuts HBM
                        # embedding traffic 3x (HBM was the binding resource).
                        t0 = p_e.tile([YT, C, xc + 2], F32, tag="E0")
                        nc.sync.dma_start(
                            out=t0[:],
                            in_=emb[:, ys + 1 : ys + 1 + YT, xs : xs + xc + 2].rearrange(
                                "c y x -> y c x"
                            ),
                        )
                        tm = p_e.tile([YT, C, xc + 2], F32, tag="E-1")
                        nc.sync.dma_start(out=tm[1:YT], in_=t0[0 : YT - 1])
                        nc.sync.dma_start(
                            out=tm[0:1],
                            in_=emb[:, ys : ys + 1, xs : xs + xc + 2].rearrange(
                                "c y x -> y c x"
                            ),
                        )
                        tp = p_e.tile([YT, C, xc + 2], F32, tag="E1")
                        nc.sync.dma_start(out=tp[0 : YT - 1], in_=t0[1:YT])
                        nc.sync.dma_start(
                            out=tp[YT - 1 : YT],
                            in_=emb[
                                :, ys + YT + 1 : ys + YT + 2, xs : xs + xc + 2
                            ].rearrange("c y x -> y c x"),
                        )
                        E = {-1: tm, 0: t0, 1: tp}

                    acc = p_acc.tile([YT, C, xc], F32, tag="acc")
                    tmp = p_tmp.tile([YT, C, xc], F32, tag="tmp")

                    def tap(k):
                        oy, ox = OFFSETS[k]
                        wk = Wt[:, k, :][:, None, :].to_broadcast((YT, C, xc))
                        return wk, E[oy][:, :, 1 + ox : 1 + ox + xc]

                    n_dve_taps = K - (dma_folds + 1 if dma_folds else 0)
                    if dma_folds:
                        # tail taps: DVE mults into separate tiles, folded into
                        # acc by the (otherwise idle) DMA engines via accum_op.
                        folds = []
                        for j in range(dma_folds + 1):
                            ft = p_acc.tile([YT, C, xc], F32, tag=f"fold{j}")
                            wk, ek = tap(n_dve_taps + j)
                            nc.vector.tensor_tensor(ft[:], wk, ek, OP.mult)
                            folds.append(ft)
                        for j in range(1, dma_folds + 1):
                            nc.gpsimd.dma_start(
                                out=folds[0][:], in_=folds[j][:], accum_op=OP.add
                            )
                    for k in range(n_dve_taps):
                        wk, ek = tap(k)
                        if k == 0:
                            nc.vector.tensor_tensor(acc[:], wk, ek, OP.mult)
                        else:
                            nc.vector.tensor_tensor(tmp[:], wk, ek, OP.mult)
                            nc.vector.tensor_tensor(acc[:], acc[:], tmp[:], OP.add)
                    if dma_folds:
                        nc.gpsimd.dma_start(
                            out=acc[:], in_=folds[0][:], accum_op=OP.add
                        )

                    nc.sync.dma_start(
                        out=out[:, ys : ys + YT, xs : xs + xc].rearrange(
                            "c y x -> y c x"
                        ),
                        in_=acc[:],
                    )

    if split_waits:
        _split_multi_waits(nc)
    return nc


_nc_cache = None


def _get_nc():
    global _nc_cache
    if _nc_cache is None:
        _nc_cache = build_nc(dma_folds=2)
    return _nc_cache


def shard_inputs(affinities, embedding):
    """Full inputs -> 8 per-core input maps (batch x H-half, halo pre-clamped)."""
    affinities = np.asarray(affinities)
    embedding = np.asarray(embedding)
    ycl = lambda idx: np.clip(idx, 0, H - 1)
    xcl = np.clip(np.arange(-1, W + 1), 0, W - 1)
    in_maps = []
    for i in range(N_CORES):
        b, half = i // 2, i % 2
        y0 = half * HH
        aff_s = np.ascontiguousarray(affinities[b, :, y0 : y0 + HH, :])
        rows = ycl(np.arange(y0 - 1, y0 + HH + 1))
        emb_s = np.ascontiguousarray(embedding[b][:, rows][:, :, xcl])
        in_maps.append({"aff": aff_s, "emb": emb_s})
    return in_maps


def unshard_outputs(results):
    out = np.empty((B, C, H, W), np.float32)
    for i in range(N_CORES):
        b, half = i // 2, i % 2
        y0 = half * HH
        out[b, :, y0 : y0 + HH, :] = results[i]["out"]
    return out


def kernel(affinities, embedding):
    nc = _get_nc()
    in_maps = shard_inputs(affinities, embedding)
    try:
        res = run_bass_kernel_spmd(nc, in_maps, list(range(N_CORES)))
    except Exception:
        # transient device errors (e.g. NRT_EXEC_UNIT_UNRECOVERABLE after an
        # earlier crashed run) usually clear on retry
        import time as _t

        _t.sleep(2.0)
        res = run_bass_kernel_spmd(nc, in_maps, list(range(N_CORES)))
    out = unshard_outputs(res.results)
    kernel.last_result = res
    return out



# revision 25
# speedup vs baseline: 2.6416x; 2.6416x over previous
"""AffinityBasedAveraging Trainium2 kernel (v3: fp16, y-major, PE-accumulate).

Computes, for affinities [B,9,H,W] and embedding [B,C,H,W]:
    w = softmax(affinities, axis=1)  (then redundant L1-normalize == no-op)
    out[b,c,y,x] = sum_k w[b,k,y,x] * embedding[b,c,clip(y+oy_k),clip(x+ox_k)]

Sharding: 8 cores = 4 batches x 2 H-halves. Host pre-transposes each
core's slabs to y-major so every DMA descriptor covers a whole
partition line: aff [256,9,512], emb [258,16,514] (1-px replicate halo
pre-clamped), out [256,16,512].

Per-core schedule, 2 y-tiles of 128 rows x full 512-col width:
  A    <- gpsimd cast-DMA aff f32->fp16   [128; 9,512]   (128 descs)
  X    = exp(A) in place                  (ScalarE, fp16)
  S    = pairwise k-sum tree              (DVE fp16 adds, 2x mode)
  R    = 1/S                              (DVE)
  W    = X * R in place (bcast over k)    (DVE fp16)
  E_oy <- gpsimd cast-DMA emb rows shifted oy in {-1,0,1}, f32->fp16
          [128; 16,514]   (3x HBM read, fp16-wide single-run descriptors)
  per 128-col chunk (4):
    P_k = W_k (bcast over c) * E_{oy(k)} shifted ox  (DVE fp16 2x mode;
          ~2 taps/chunk on the Pool engine to balance)
    acc = sum_k P_k   (PE identity-matmul accumulate into PSUM f32)
    osb = copy(acc)   (ScalarE PSUM->SBUF f32)
    out <- dma osb
Engine balance per core (cost model): DVE ~80us (9 mults/px + softmax),
Pool ~77us (tap mults + SWDGE desc-gen), PE ~66us (all adds),
DMA ~65us, Act ~24us.
"""

import numpy as np

import bass_rust
import concourse.bass as bass
import concourse.mybir as mybir
import concourse.tile as tile
from concourse import masks
from concourse.bass_utils import run_bass_kernel_spmd

F32 = mybir.dt.float32
F16 = mybir.dt.float16
AF = mybir.ActivationFunctionType
OP = mybir.AluOpType
AX = mybir.AxisListType

B, C, H, W = 4, 16, 512, 512
K = 9
OFFSETS = [(-1, -1), (-1, 0), (-1, 1), (0, -1), (0, 0), (0, 1), (1, -1), (1, 0), (1, 1)]
N_CORES = 8
HH = H // 2          # rows per core (256)
YT = 128             # y-tile rows (partition dim)
XC = 128             # x-chunk cols (product/matmul granularity)

_wsplit_ctr = [0]


def _split_multi_waits(nc):
    """This container's walrus rejects >1 semaphore wait per instruction
    ("Too many sync wait commands"). Split extra waits into same-engine
    NoOp prefixes."""
    n = 0
    for f in nc.m.functions:
        for bb in f.blocks:
            insts = bb.instructions
            if not any(
                i.sync_info is not None and len(i.sync_info.on_wait or []) > 1
                for i in insts
            ):
                continue
            new = []
            for inst in insts:
                si = inst.sync_info
                waits = list(si.on_wait) if si is not None and si.on_wait else []
                if len(waits) > 1:
                    for w in waits[:-1]:
                        _wsplit_ctr[0] += 1
                        nop = mybir.InstNoOp(name=f"I-wsplit-{_wsplit_ctr[0]}")
                        nop.engine = inst.engine
                        nop.sync_info = bass_rust.SyncInfo(on_wait=[w], on_update=[])
                        new.append(nop)
                        n += 1
                    inst.sync_info = bass_rust.SyncInfo(
                        on_wait=[waits[-1]], on_update=list(si.on_update or [])
                    )
                new.append(inst)
            insts[:] = new
    return n


def build_nc(
    split_waits=True,
    reps=1,
    pool_taps=(4, 8),
    pool_taps2=(4,),
    mm_banks=1,
    pool_adds=(),
    load_pos=2,
):
    """pool_taps / pool_taps2: tap indices computed on the Pool engine for
    even / odd chunks (averaging a non-integer tap count per chunk).
    mm_banks: PSUM banks covered by one matmul (1 = 4 matmuls/tap into
    separate single-bank accs; walrus rejects >512 f32 per matmul out).
    pool_adds: (a, b) tap pairs pre-summed on the Pool engine so PE skips
    tap b's matmuls (PE stationary reloads make PE co-critical on HW)."""
    nc = bass.Bass("TRN2", target_bir_lowering=False, debug=False, num_devices=N_CORES)
    aff = nc.declare_dram_parameter("aff", [HH, K, W], F32, isOutput=False)
    emb = nc.declare_dram_parameter("emb", [HH + 2, C, W + 2], F32, isOutput=False)
    out = nc.declare_dram_parameter("out", [HH, C, W], F32, isOutput=True)

    n_bank = (C * XC * 4) // 2048  # PSUM banks per chunk acc (4)
    cpb = C // n_bank              # channels per bank (4)

    lp = nc.allow_low_precision(
        reason="fp16 softmax weights/taps; PE accumulates products in f32 "
        "PSUM; target gate is rel_err < 2e-2"
    )
    with lp, tile.TileContext(nc) as tc:
        with (
            tc.tile_pool(name="p_const", bufs=1) as p_const,
            tc.tile_pool(name="p_a", bufs=2) as p_a,
            tc.tile_pool(name="p_s", bufs=2) as p_s,
            tc.tile_pool(name="p_e", bufs=2) as p_e,
            tc.tile_pool(name="p_prod", bufs=1) as p_prod,
            tc.tile_pool(name="p_o", bufs=2) as p_o,
            tc.tile_pool(name="p_ps", bufs=2, space="PSUM") as p_ps,
        ):
            ident = p_const.tile([YT, YT], F16, tag="ident")
            masks.make_identity(nc, ident[:])
            # Warm the Exp activation table while the first DMAs run.
            warm = p_const.tile([1, 1], F16, tag="warm")
            nc.scalar.activation(warm[:], ident[0:1, 0:1], AF.Exp)

            tiles = [ty for _ in range(reps) for ty in range(HH // YT)]

            def emit_loads(ys):
                A = p_a.tile([YT, K, W], F16, tag="A")
                nc.gpsimd.dma_start(out=A[:], in_=aff[ys : ys + YT, :, :])
                E = {}
                for oy in (-1, 0, 1):
                    t = p_e.tile([YT, C, W + 2], F16, tag=f"E{oy}")
                    rs = ys + oy + 1
                    nc.gpsimd.dma_start(out=t[:], in_=emb[rs : rs + YT, :, :])
                    E[oy] = t
                return A, E

            # software-pipeline the loads one tile ahead: the Pool queue is
            # in-order, so tile t+1's SWDGE descriptor-gens are emitted in
            # the middle of tile t's compute (after chunk `load_pos`'s pool
            # taps) — early enough that the E transfers land before tile
            # t+1 needs them, late enough not to delay tile t's first
            # chunks' pool taps.
            loaded = emit_loads(tiles[0] * YT)
            for ti in range(len(tiles)):
                    A, E = loaded
                    ys = tiles[ti] * YT

                    # X = exp(A) in place (two halves so the k-sum can start
                    # early); then pairwise k-sum tree (fp16 2x-mode adds
                    # beat a single tensor_reduce ~2x here).
                    nc.scalar.activation(A[:, 0:4], A[:, 0:4], AF.Exp)
                    nc.scalar.activation(A[:, 4:K], A[:, 4:K], AF.Exp)
                    sa = p_s.tile([YT, 4, W], F16, tag="sa")
                    nc.vector.tensor_tensor(
                        sa[:], A[:, 0:8:2, :], A[:, 1:8:2, :], OP.add
                    )
                    sb = p_s.tile([YT, 2, W], F16, tag="sb")
                    nc.vector.tensor_tensor(
                        sb[:], sa[:, 0:4:2, :], sa[:, 1:4:2, :], OP.add
                    )
                    sc = p_s.tile([YT, W], F16, tag="sc")
                    nc.vector.tensor_tensor(sc[:], sb[:, 0, :], sb[:, 1, :], OP.add)
                    S = p_s.tile([YT, W], F16, tag="S")
                    nc.vector.tensor_tensor(S[:], sc[:], A[:, 8, :], OP.add)
                    R = p_s.tile([YT, W], F16, tag="R")
                    nc.vector.reciprocal(R[:], S[:])
                    # W = X * R in place (bcast over k)
                    nc.vector.tensor_tensor(
                        A[:], A[:], R[:, None, :].to_broadcast((YT, K, W)), OP.mult
                    )

                    for xh in range(W // XC):
                        if xh == load_pos and ti + 1 < len(tiles):
                            loaded = emit_loads(tiles[ti + 1] * YT)
                        xs = xh * XC
                        ptaps = pool_taps if xh % 2 == 0 else pool_taps2

                        prods = []
                        for k in range(K):
                            oy, ox = OFFSETS[k]
                            wk = A[:, k, xs : xs + XC][:, None, :].to_broadcast(
                                (YT, C, XC)
                            )
                            ek = E[oy][:, :, 1 + ox + xs : 1 + ox + xs + XC]
                            P = p_prod.tile([YT, C, XC], F16, tag=f"P{k}")
                            eng = nc.gpsimd if k in ptaps else nc.vector
                            eng.tensor_tensor(P[:], wk, ek, OP.mult)
                            prods.append(P)
                        skip = set()
                        for a, bb_ in pool_adds:
                            # merge tap bb_ into tap a on Pool; PE then skips
                            # bb_'s matmuls
                            nc.gpsimd.tensor_tensor(
                                prods[a][:], prods[a][:], prods[bb_][:], OP.add
                            )
                            skip.add(bb_)
                        mm_ks = [k for k in range(K) if k not in skip]

                        # one PSUM tile per mm_banks-bank group so each
                        # group's copy/store can fire as soon as its own
                        # stop matmul lands (whole-tile dep tracking would
                        # otherwise serialize them behind all matmuls)
                        n_grp = n_bank // mm_banks
                        cpg = C // n_grp
                        accs = []
                        for g in range(n_grp):
                            acc_g = p_ps.tile(
                                [YT, cpg, XC], F32, tag=f"acc{g}", name=f"acc{g}"
                            )
                            accs.append(acc_g)
                        for k in mm_ks:
                            for g in range(n_grp):
                                nc.tensor.matmul(
                                    accs[g][:],
                                    ident[:],
                                    prods[k][:, g * cpg : (g + 1) * cpg, :],
                                    start=(k == mm_ks[0]),
                                    stop=(k == mm_ks[-1]),
                                )
                        for g in range(n_grp):
                            osb = p_o.tile([YT, cpg, XC], F32, tag=f"osb{g}")
                            nc.scalar.activation(osb[:], accs[g][:], AF.Copy)
                            nc.sync.dma_start(
                                out=out[
                                    ys : ys + YT,
                                    g * cpg : (g + 1) * cpg,
                                    xs : xs + XC,
                                ],
                                in_=osb[:],
                            )

    if split_waits:
        _split_multi_waits(nc)
    return nc


_nc_cache = None


def _get_nc():
    global _nc_cache
    if _nc_cache is None:
        _nc_cache = build_nc()
    return _nc_cache


def shard_inputs(affinities, embedding):
    """Full inputs -> 8 per-core y-major input maps (batch x H-half,
    1-px replicate halo pre-clamped)."""
    affinities = np.asarray(affinities)
    embedding = np.asarray(embedding)
    ycl = lambda idx: np.clip(idx, 0, H - 1)
    xcl = np.clip(np.arange(-1, W + 1), 0, W - 1)
    in_maps = []
    for i in range(N_CORES):
        b, half = i // 2, i % 2
        y0 = half * HH
        aff_s = np.ascontiguousarray(
            affinities[b, :, y0 : y0 + HH, :].transpose(1, 0, 2)
        )
        rows = ycl(np.arange(y0 - 1, y0 + HH + 1))
        emb_s = np.ascontiguousarray(
            embedding[b][:, rows][:, :, xcl].transpose(1, 0, 2)
        )
        in_maps.append({"aff": aff_s, "emb": emb_s})
    return in_maps


def unshard_outputs(results):
    out = np.empty((B, C, H, W), np.float32)
    for i in range(N_CORES):
        b, half = i // 2, i % 2
        y0 = half * HH
        out[b, :, y0 : y0 + HH, :] = results[i]["out"].transpose(1, 0, 2)
    return out


def kernel(affinities, embedding):
    nc = _get_nc()
    in_maps = shard_inputs(affinities, embedding)
    try:
        res = run_bass_kernel_spmd(nc, in_maps, list(range(N_CORES)))
    except Exception:
        # transient device errors (e.g. NRT_EXEC_UNIT_UNRECOVERABLE after an
        # earlier crashed run) usually clear on retry
        import time as _t

        _t.sleep(2.0)
        res = run_bass_kernel_spmd(nc, in_maps, list(range(N_CORES)))
    out = unshard_outputs(res.results)
    kernel.last_result = res
    return out
